# revision 35
# baseline (speedup 1.0000x reference)
"""APPNP GNN kernel for 8 Trainium2 NeuronCores.

h = 0.8 * D_in^{-1/2} A D_out^{-1/2} (X W^T + b) + 0.2 * (X W^T + b)

Strategy: dst-shard nodes across 8 cores. Each core computes h0 for its
own nodes (PE matmul, bf16), scales by rsqrt(out-degree), AllGathers the
bf16 table in 4 chunks, then per-edge dma_gathers source rows from HBM
and segment-reduces them into per-dst-window PSUM tiles via one-hot
matmuls (S^T @ msgs).

Pipeline decoupling: passes run pass-major, window-major within each
pass, with pass 0 overlapping phase 1 / the AllGathers. Each pass has
its own bf16 accumulator slab; finished window psums are drained there
by the SCALAR engine only, so no mid-stream op on Vector ever waits on
fresh matmul results -- Vector does nothing but the batched IS_EQ
one-hot builds and runs ahead. A single batched endgame sums the four
accumulators, applies norm_in, and blends into 0.2*h0 (all bf16;
output cast to fp32 on host).

Perf note (measured): the wall is the per-edge dma_gather descriptor
stream itself -- gather-only is ~1.95 ms of the ~2.04 ms total.
Per-descriptor cost has a ~2.6 ns/desc aggregate mechanics floor
(sequential-index probe: 1.08 ms) plus a random-access HBM latency
adder (+0.87 ms). SBUF-source transpose-mode gathers measured
~3.35 ns/desc (1.40 ms) but need a per-tile PE re-transpose that
wrecks the pipeline. Descriptor count = E/core is irreducible, so
~1.95 ms is the practical floor for this gather-based design.
"""

import os
import sys

sys.path.insert(0, "/opt/trn_rl_repo")

SKIP_EDGES = os.environ.get("K_SKIP_EDGES", "") == "1"
GATHER_ONLY = os.environ.get("K_GATHER_ONLY", "") == "1"
FAKE_IDX = os.environ.get("K_FAKE_IDX", "") == "1"
SBUF_GATHER_PROBE = os.environ.get("K_SBUF_PROBE", "") == "1"
SINGLE_PACKET = os.environ.get("K_SINGLE_PACKET", "0") == "1"
SLIPS = tuple(
    int(x) for x in os.environ.get("K_SLIPS", "3,5,7").split(",")
)
SBATCH = int(os.environ.get("K_SBATCH", "16"))  # S one-hots per DVE op
WG = int(os.environ.get("K_WG", "16"))  # windows per psum group
PSW = 8  # windows per PSUM bank tile
GPOOL = int(os.environ.get("K_GPOOL", "7"))
SPOOL = int(os.environ.get("K_SPOOL", "4"))
NQUEUE = int(os.environ.get("K_NQUEUE", "4"))

import numpy as np
import ml_dtypes

import concourse.bass as bass
import concourse.bacc as bacc
import concourse.tile as tile
import concourse.mybir as mybir
from concourse.bass_utils import run_bass_kernel_spmd

F32 = mybir.dt.float32
BF16 = mybir.dt.bfloat16
I16 = mybir.dt.int16
I32 = mybir.dt.int32

NCORES = 8
SLAB_TILES = int(os.environ.get("K_SLAB", "32"))  # max edge tiles per dma_gather
CHT = 256        # idx chunk size in tiles
ALPHA = 0.2
TPB = 4  # node tiles per phase-1 matmul group


def _cfg(N, F, C):
    sh = (N + NCORES - 1) // NCORES          # nodes per core
    shp = ((sh + 127) // 128) * 128          # padded to 128
    nw = shp // 128                          # dst windows per core
    # split each core's shard into NPASS quarters (tile-aligned to TPB) so
    # AllGather q can fire as soon as phase 1 finishes quarter q
    npass = 4
    # graduated quarters: small starter so AllGather 0 fires early;
    # chunk 1 small enough that its (contended) transfer completes
    # before pass 0's gathers run dry
    q0, q1 = 12, 26
    base = (nw - q0 - q1) // (npass - 2)
    q_tiles = [q0, q1] + [base] * (npass - 3) + [
        nw - q0 - q1 - base * (npass - 3)
    ]
    q_rows = [t * 128 for t in q_tiles]
    q_off = list(np.cumsum([0] + q_rows[:-1]))
    chunk_rows = [NCORES * r for r in q_rows]          # table rows per chunk
    assert max(chunk_rows) < 32768, chunk_rows
    return dict(N=N, F=F, C=C, SH=sh, SHP=shp, NW=nw, NPASS=npass,
                CHUNK_ROWS=chunk_rows, Q_TILES=q_tiles, Q_ROWS=q_rows,
                Q_OFF=q_off)


def _tile_schedule(t_pw, NW, NPASS):
    """Stream order: pass-major, window-major within pass. Returns
    tile_order [(p, w)], seg_pos {(p,w): first tile}, and the slab
    list [(t0, nts, pass)]."""
    tile_order = []
    for p in range(NPASS):
        for w in range(NW):
            tile_order += [(p, w)] * int(t_pw[p][w])
    ntiles = len(tile_order)
    seg_pos = {}
    prev = None
    for t, pw in enumerate(tile_order):
        if pw != prev:
            seg_pos[pw] = t
            prev = pw
    slabs = []
    t0 = 0
    while t0 < ntiles:
        p = tile_order[t0][0]
        end = min(t0 + SLAB_TILES, ntiles, (t0 // CHT + 1) * CHT)
        e = t0
        while e < end and tile_order[e][0] == p:
            e += 1
        slabs.append((t0, e - t0, p))
        t0 = e
    return tile_order, seg_pos, slabs


def _host_prep(in_feat, W, b, src, dst, cfg):
    """Shard + reformat inputs; build core-uniform edge-tile structure."""
    N, F, C = cfg["N"], cfg["F"], cfg["C"]
    SH, SHP, NW, NPASS = cfg["SH"], cfg["SHP"], cfg["NW"], cfg["NPASS"]

    src = np.asarray(src, dtype=np.int64)
    dst = np.asarray(dst, dtype=np.int64)

    # chunk q = quarter q of EVERY core's shard (so AllGather q only needs
    # phase-1 quarter q); within chunk q rows are core-major
    q_rows = np.asarray(cfg["Q_ROWS"], dtype=np.int64)
    q_off = np.asarray(cfg["Q_OFF"], dtype=np.int64)
    score = src // SH
    slocal = src % SH
    passno = (np.searchsorted(q_off, slocal, side="right") - 1).astype(np.int64)
    idx16 = (score * q_rows[passno] + (slocal - q_off[passno])).astype(np.int32)
    passno = passno.astype(np.int32)
    core = (dst // SH).astype(np.int32)
    dloc = (dst % SH).astype(np.int64)
    wno = (dloc // 128).astype(np.int32)
    drel = (dloc % 128).astype(np.int32)

    # per-(core, pass, window) counts -> uniform tile structure
    key = (core.astype(np.int64) * NPASS + passno) * NW + wno
    counts = np.bincount(key, minlength=NCORES * NPASS * NW).reshape(
        NCORES, NPASS, NW
    )
    t_pw = np.maximum((counts + 127) // 128, 1).max(axis=0)  # [NPASS, NW]

    tile_order, seg_pos, slabs = _tile_schedule(t_pw, NW, NPASS)
    ntiles = len(tile_order)

    # per-core streams
    in_feat = np.asarray(in_feat, dtype=np.float32)
    WT = np.ascontiguousarray(
        np.asarray(W, dtype=np.float32).T.astype(ml_dtypes.bfloat16)
    )  # [F, C] bf16
    bias = np.asarray(b, dtype=np.float32).reshape(C, 1)
    ident = np.eye(C, dtype=np.float32)

    deg_out = np.bincount(src, minlength=N)
    deg_in = np.bincount(dst, minlength=N)
    rp_out = np.concatenate([[0], np.cumsum(deg_out)])
    rp_in = np.concatenate([[0], np.cumsum(deg_in)])

    iota = np.tile(np.arange(128, dtype=np.float32), (128, SBATCH)).astype(
        ml_dtypes.bfloat16
    )  # [128, SBATCH*128]

    order = np.lexsort((idx16, wno, passno, core))
    so_pass, so_idx, so_rel = passno[order], idx16[order], drel[order]
    so_core, so_w = core[order], wno[order]
    # edge range per (core, pass, window) in sorted order
    seg_start = np.searchsorted(
        (so_core.astype(np.int64) * NPASS + so_pass) * NW + so_w,
        np.arange(NCORES * NPASS * NW + 1),
    ).reshape(-1)

    NBLK = (NW + TPB - 1) // TPB
    in_maps = []
    for k in range(NCORES):
        inT = np.zeros((F, SHP), dtype=np.float32)
        lo, hi = k * SH, min((k + 1) * SH, N)
        inT[:, : hi - lo] = in_feat[lo:hi].T
        # [p, blk, c, n]: partition-contiguous per phase-1 block DMA
        inT_p = np.zeros((F, NBLK * TPB * 128), dtype=np.float32)
        inT_p[:, :SHP] = inT
        v = inT_p.reshape(F // 128, 128, NBLK, TPB * 128)
        inT_t = np.ascontiguousarray(v.transpose(1, 2, 0, 3)).astype(
            ml_dtypes.bfloat16
        )

        def rp_mats(rp):
            v = rp[lo : hi + 1]
            v = np.concatenate([v, np.full(SHP + 1 - len(v), v[-1], v.dtype)])
            lo_m = v[:SHP].reshape(NW, 128).T.astype(np.int32)
            hi_m = v[1 : SHP + 1].reshape(NW, 128).T.astype(np.int32)
            return np.ascontiguousarray(lo_m), np.ascontiguousarray(hi_m)

        rpo_lo, rpo_hi = rp_mats(rp_out)
        rpi_lo, rpi_hi = rp_mats(rp_in)

        idx_stream = np.zeros(ntiles * 128, dtype=np.int16)
        rel_stream = np.full(ntiles * 128, -1.0, dtype=np.float32)
        for p in range(NPASS):
            for w in range(NW):
                s0 = seg_start[(k * NPASS + p) * NW + w]
                s1 = seg_start[(k * NPASS + p) * NW + w + 1]
                off = seg_pos[(p, w)] * 128
                idx_stream[off : off + (s1 - s0)] = so_idx[s0:s1]
                rel_stream[off : off + (s1 - s0)] = so_rel[s0:s1]

        if FAKE_IDX:
            # sequential indices: perfect-locality gather experiment
            idx_stream = (np.arange(ntiles * 128) % 24576).astype(np.int16)
        if SBUF_GATHER_PROBE:
            rng = np.random.RandomState(0)
            idx_stream = rng.randint(
                0, 12544, size=ntiles * 128
            ).astype(np.int16)
        idx_w = np.tile(
            np.ascontiguousarray(idx_stream.reshape(-1, 16).T), (8, 1)
        )  # [128, ntiles*8]: 16-part wrap replicated per Q7 core
        rel_m = np.ascontiguousarray(
            rel_stream.reshape(ntiles, 128).T.astype(ml_dtypes.bfloat16)
        )  # [128, ntiles]

        in_maps.append(
            {
                "inT": inT_t,
                "wt": WT,
                "bias": bias,
                "ident": ident,
                "iota": iota,
                "rpo_lo": rpo_lo,
                "rpo_hi": rpo_hi,
                "rpi_lo": rpi_lo,
                "rpi_hi": rpi_hi,
                "idx": idx_w,
                "rel": rel_m,
            }
        )

    struct = dict(t_pw=t_pw, ntiles=ntiles, tile_order=tile_order,
                  seg_pos=seg_pos, slabs=slabs)
    return in_maps, struct


def _build_program(cfg, struct):
    F, C = cfg["F"], cfg["C"]
    SHP, NW, NPASS = cfg["SHP"], cfg["NW"], cfg["NPASS"]
    CHUNK_ROWS = cfg["CHUNK_ROWS"]
    t_pw, ntiles = struct["t_pw"], struct["ntiles"]
    tile_order, seg_pos, slabs = (
        struct["tile_order"], struct["seg_pos"], struct["slabs"],
    )
    KC = F // 128  # contraction chunks in phase 1

    nc = bacc.Bacc(
        "TRN2", target_bir_lowering=False, debug=False, num_devices=NCORES,
        num_swdge_queues=4,
    )

    NBLK = (NW + 3) // 4
    inT_d = nc.dram_tensor(
        "inT", [128, NBLK, F // 128, 4 * 128], BF16, kind="ExternalInput"
    ).ap()
    wt_d = nc.dram_tensor("wt", [F, C], BF16, kind="ExternalInput").ap()
    bias_d = nc.dram_tensor("bias", [C, 1], F32, kind="ExternalInput").ap()
    ident_d = nc.dram_tensor("ident", [C, C], F32, kind="ExternalInput").ap()
    iota_d = nc.dram_tensor(
        "iota", [128, SBATCH * 128], BF16, kind="ExternalInput"
    ).ap()
    rp_d = {
        n: nc.dram_tensor(n, [128, NW], I32, kind="ExternalInput").ap()
        for n in ("rpo_lo", "rpo_hi", "rpi_lo", "rpi_hi")
    }
    idx_d = nc.dram_tensor(
        "idx", [128, ntiles * 8], I16, kind="ExternalInput"
    ).ap()
    rel_d = nc.dram_tensor("rel", [128, ntiles], BF16, kind="ExternalInput").ap()
    hout_d = nc.dram_tensor("hout", [SHP, C], BF16, kind="ExternalOutput").ap()

    with tile.TileContext(nc) as tc:
        with (
            tc.tile_pool(name="const", bufs=1) as cpool,
            tc.tile_pool(name="bigbuf", bufs=1) as bpool,
            tc.tile_pool(name="inT", bufs=3) as ipool,
            tc.tile_pool(name="gat", bufs=GPOOL) as gpool,
            tc.tile_pool(name="idxs", bufs=3) as idxpool,
            tc.tile_pool(name="sbuild", bufs=SPOOL) as spool,
            tc.tile_pool(name="ps1", bufs=2, space="PSUM") as ps1,
            tc.tile_pool(name="pse", bufs=4, space="PSUM") as pse,
            tc.tile_pool(name="dram", bufs=1, space="DRAM") as dpool,
        ):
            # ---- load constants ----
            wt_s = cpool.tile([128, KC, C], BF16, tag="wt")
            nc.sync.dma_start(
                wt_s[:], wt_d.rearrange("(c p) f -> p c f", p=128)
            )
            bias_s = cpool.tile([C, 1], F32, tag="bias")
            nc.sync.dma_start(bias_s[:], bias_d)
            ident_s = cpool.tile([C, C], F32, tag="ident")
            nc.sync.dma_start(ident_s[:], ident_d)
            iota_s = cpool.tile([128, SBATCH * 128], BF16, tag="iota")
            nc.sync.dma_start(iota_s[:], iota_d)
            rel_s = cpool.tile([128, ntiles], BF16, tag="rel")
            nc.sync.dma_start(rel_s[:], rel_d)
            rp_s = {}
            for n in rp_d:
                rp_s[n] = cpool.tile([128, NW], I32, tag=n, name=n)
                nc.sync.dma_start(rp_s[n][:], rp_d[n])

            # ---- degree norms: norm = sqrt(scale / clip(deg, 1)) ----
            def make_norm(lo, hi, scale, tag):
                deg = cpool.tile([128, NW], F32, tag=tag + "_deg")
                nc.vector.tensor_tensor(
                    deg[:], hi[:], lo[:], op=mybir.AluOpType.subtract
                )
                nc.vector.tensor_scalar_max(deg[:], deg[:], 1.0)
                rec = cpool.tile([128, NW], F32, tag=tag + "_rec")
                nc.vector.reciprocal(rec[:], deg[:])
                norm = cpool.tile([128, NW], F32, tag=tag)
                nc.scalar.activation(
                    norm[:], rec[:], mybir.ActivationFunctionType.Sqrt,
                    scale=scale,
                )
                return norm

            norm_out = make_norm(rp_s["rpo_lo"], rp_s["rpo_hi"], 1.0, "nout")
            norm_in = make_norm(
                rp_s["rpi_lo"], rp_s["rpi_hi"], (1.0 - ALPHA) ** 2, "nin"
            )

            # ---- big SBUF buffers ----
            h0s_s = bpool.tile([128, NW, 128], BF16, tag="h0s")  # padded table
            nc.vector.memset(h0s_s[:, :, C:128], 0.0)
            h0b_s = bpool.tile([128, NW, C], BF16, tag="h0b")  # 0.2*h0 -> out
            acc4_s = bpool.tile([128, NPASS * NW, C], BF16, tag="acc4")

            # ---- phase 1 ----
            coll_ins = []
            h0s_dram = dpool.tile([SHP, 128], BF16, tag="h0s_dram",
                                  name="h0s_dram")
            tables = [
                dpool.tile([CHUNK_ROWS[q], 128], BF16, tag=f"table{q}",
                           name=f"table{q}", addr_space="Shared")
                for q in range(NPASS)
            ]
            Q_TILES = cfg["Q_TILES"]
            qt_off = list(np.cumsum([0] + Q_TILES[:-1]))

            qwrite_ins = {}

            def emit_quarter_write(q):
                a, b = qt_off[q], qt_off[q] + Q_TILES[q]
                di = nc.sync.dma_start(
                    h0s_dram[a * 128:b * 128].rearrange(
                        "(t p) f -> p t f", p=128
                    ),
                    h0s_s[:, a:b, :],
                )
                qwrite_ins[q] = di

            def emit_quarter_coll(q):
                a, b = qt_off[q], qt_off[q] + Q_TILES[q]
                ci = nc.gpsimd.collective_compute(
                    "AllGather",
                    mybir.AluOpType.bypass,
                    replica_groups=[list(range(NCORES))],
                    ins=[h0s_dram[a * 128:b * 128].opt()],
                    outs=[tables[q][:].opt()],
                )
                # DRAM-space deps are not auto-tracked: explicitly order the
                # collective after its quarter's h0s_dram write completes
                tile.add_dep_helper(ci.ins, qwrite_ins[q].ins, sync=True)
                coll_ins.append(ci.ins)

            def emit_phase1():
                nq = 0
                for g0 in range(0, NW, TPB):
                    nb = min(TPB, NW - g0)
                    t = ipool.tile([128, KC, TPB * 128], BF16, tag="inT",
                                   name="t")
                    nc.sync.dma_start(t[:], inT_d[:, g0 // TPB, :, :])
                    psT = ps1.tile([C, TPB * 128], F32, tag="psT", name="psT")
                    for c in range(KC):
                        nc.tensor.matmul(
                            psT[:, : nb * 128],
                            lhsT=wt_s[:, c, :],
                            rhs=t[:, c, : nb * 128],
                            start=(c == 0),
                            stop=(c == KC - 1),
                        )
                    h0T = ipool.tile([C, TPB * 128], F32, tag="h0T",
                                     name="h0T")
                    nc.vector.tensor_scalar(
                        h0T[:, : nb * 128], psT[:, : nb * 128], bias_s[:],
                        None, op0=mybir.AluOpType.add,
                    )
                    for j in range(nb):
                        tt = g0 + j
                        pst = ps1.tile([128, C], F32, tag="pst", name="pst")
                        nc.tensor.transpose(
                            pst[:], h0T[:, j * 128 : (j + 1) * 128], ident_s[:]
                        )
                        nc.scalar.activation(
                            h0s_s[:, tt, 0:C], pst[:],
                            mybir.ActivationFunctionType.Copy,
                            scale=norm_out[:, tt : tt + 1],
                        )
                        nc.scalar.activation(
                            h0b_s[:, tt, :], pst[:],
                            mybir.ActivationFunctionType.Copy, scale=ALPHA,
                        )
                    while nq < NPASS and g0 + nb >= qt_off[nq] + Q_TILES[nq]:
                        emit_quarter_write(nq)
                        if nq == 0:
                            emit_quarter_coll(0)
                        nq += 1
                assert nq == NPASS, (nq, NPASS)

            # ---- SBUF-gather probe: copy part of chunk 0 into SBUF
            # and run transpose-mode gathers from it ----
            tbl_sbuf = None
            if SBUF_GATHER_PROBE:
                tbl_sbuf = bpool.tile([128, 16384], BF16, tag="tblsb")
                nc.sync.dma_start(
                    tbl_sbuf[:, : 98 * 128],
                    tables[0][0:12544].rearrange(
                        "(p t) f -> p (t f)", p=128
                    ),
                )

            # ---- edge phase ----
            st = dict(s_cur=None, nslab=0)
            ps_by_w = {}
            seen_tbl = set()

            def load_idx_chunk(c0):
                # chunk covers tiles [c0, c0+CHT)
                it = idxpool.tile([128, CHT * 8], I16, tag="idxs",
                                  name=f"it{c0}")
                ncols = min(CHT, ntiles - c0) * 8
                nc.sync.dma_start(
                    it[:, :ncols], idx_d[:, c0 * 8 : c0 * 8 + ncols]
                )
                return it

            idx_chunks = {}
            if not SKIP_EDGES:
                idx_chunks[0] = load_idx_chunk(0)
                if ntiles > CHT:
                    idx_chunks[CHT] = load_idx_chunk(CHT)


            def emit_slab(si, t0, nts, p):
                tbl = tables[p][:]
                # idx chunk management + prefetch 2 ahead
                c0 = (t0 // CHT) * CHT
                nxt = c0 + 2 * CHT
                if nxt < ntiles and nxt not in idx_chunks:
                    idx_chunks[nxt] = load_idx_chunk(nxt)
                it = idx_chunks[c0]
                if p == 0 and si in SLIPS:
                    q = SLIPS.index(si) + 1
                    if len(coll_ins) == q:
                        emit_quarter_coll(q)
                if SBUF_GATHER_PROBE:
                    g = gpool.tile([128, 1, SLAB_TILES * 128], BF16,
                                   tag="gat", name="g")
                else:
                    g = gpool.tile([128, SLAB_TILES, 128], BF16, tag="gat",
                                   name="g")
                nidx = nts * 128
                toff = t0 - c0
                if SBUF_GATHER_PROBE:
                    gi = nc.gpsimd.dma_gather(
                        g[:, :, :nidx],
                        tbl_sbuf[:],
                        it[:, toff * 8 : toff * 8 + nidx // 16],
                        num_idxs=nidx,
                        num_idxs_reg=nidx,
                        elem_size=128,
                        transpose=True,
                        single_packet=SINGLE_PACKET,
                        queue_num=st["nslab"] % NQUEUE,
                        sbuf_tokens_per_rank=128,
                        sbuf_free_dim_per_rank=32768,
                    )
                else:
                    gi = nc.gpsimd.dma_gather(
                        g[:, :nts, :],
                        tbl,
                        it[:, toff * 8 : toff * 8 + nidx // 16],
                        num_idxs=nidx,
                        num_idxs_reg=nidx,
                        elem_size=128,
                        single_packet=SINGLE_PACKET,
                        queue_num=st["nslab"] % NQUEUE,
                    )
                st["nslab"] += 1
                if p not in seen_tbl:
                    seen_tbl.add(p)
                    tile.add_dep_helper(gi.ins, coll_ins[p], sync=True)
                if not GATHER_ONLY:
                    for ti in range(nts):
                        emit_tile(t0, ti, p, g)

            def emit_tile(t0, ti, p, g):
                tt = t0 + ti
                if tt % SBATCH == 0:  # build S batch
                    st["s_cur"] = spool.tile(
                        [128, SBATCH * 128], BF16, tag="sb", name="s"
                    )
                    nb4 = min(SBATCH, ntiles - tt)
                    rel_b = rel_s[:, tt : tt + nb4].unsqueeze(-1)
                    nc.vector.tensor_tensor(
                        st["s_cur"][:, : nb4 * 128].rearrange(
                            "p (a b) -> p a b", b=128
                        ),
                        iota_s[:, : nb4 * 128].rearrange(
                            "p (a b) -> p a b", b=128
                        ),
                        rel_b.broadcast_to((128, nb4, 128)),
                        op=mybir.AluOpType.is_equal,
                    )
                sc = tt % SBATCH
                s_t = st["s_cur"][:, sc * 128 : (sc + 1) * 128]
                pw, w = tile_order[tt]
                assert pw == p
                tloc = tt - seg_pos[(p, w)]
                first = tloc == 0
                last = tloc == t_pw[p][w] - 1
                if first:
                    ps_by_w[w] = pse.tile([128, C], F32, tag="pse",
                                          name="ps")
                cur_ps = ps_by_w.pop(w) if last else ps_by_w[w]
                nc.tensor.matmul(
                    cur_ps[:],
                    lhsT=s_t,
                    rhs=g[:, ti, 0:C],
                    start=first,
                    stop=last,
                )
                if last:
                    # drain psum to this pass's bf16 accumulator (scalar
                    # engine only -- no mid-stream vector dependency)
                    nc.scalar.activation(
                        acc4_s[:, p * NW + w, :], cur_ps[:],
                        mybir.ActivationFunctionType.Copy,
                    )

            norm_bf = cpool.tile([128, NW], BF16, tag="nin_bf")
            nc.vector.tensor_copy(norm_bf[:], norm_in[:])

            emit_phase1()
            if not SKIP_EDGES:
                si = -1
                for t0, nts, p in slabs:
                    if p == 0:
                        si += 1
                    else:
                        while len(coll_ins) <= p:
                            emit_quarter_coll(len(coll_ins))
                    emit_slab(si if p == 0 else -1, t0, nts, p)
                # endgame: batched reduce of the 4 accumulators + blend
                a = acc4_s.rearrange("p (q w) f -> p q w f", q=NPASS)
                nc.vector.tensor_tensor(
                    a[:, 0], a[:, 0], a[:, 1], op=mybir.AluOpType.add
                )
                nc.vector.tensor_tensor(
                    a[:, 2], a[:, 2], a[:, 3], op=mybir.AluOpType.add
                )
                nc.vector.tensor_tensor(
                    a[:, 0], a[:, 0], a[:, 2], op=mybir.AluOpType.add
                )
                nb = norm_bf[:].unsqueeze(-1)
                nc.vector.tensor_tensor(
                    a[:, 0], a[:, 0], nb.broadcast_to((128, NW, C)),
                    op=mybir.AluOpType.mult,
                )
                nc.vector.tensor_tensor(
                    h0b_s[:], h0b_s[:], a[:, 0], op=mybir.AluOpType.add
                )
                nc.scalar.dma_start(
                    hout_d.rearrange("(t p) f -> p t f", p=128), h0b_s[:]
                )
            else:
                for q in range(1, NPASS):
                    emit_quarter_coll(q)
                nc.sync.dma_start(
                    hout_d.rearrange("(t p) f -> p t f", p=128), h0b_s[:]
                )
            if GATHER_ONLY:
                nc.sync.dma_start(
                    hout_d.rearrange("(t p) f -> p t f", p=128), h0b_s[:]
                )

    nc.compile()
    return nc


def run(in_feat, W, b, src, dst, trace=False):
    N, F = in_feat.shape
    C = W.shape[0]
    cfg = _cfg(N, F, C)
    in_maps, struct = _host_prep(in_feat, W, b, src, dst, cfg)
    nc = _build_program(cfg, struct)
    res = run_bass_kernel_spmd(
        nc, in_maps, list(range(NCORES)), trace=trace
    )
    outs = [res.results[k]["hout"][: cfg["SH"]] for k in range(NCORES)]
    full = np.concatenate(outs, axis=0)[:N].astype(np.float32)
    return full, res


def kernel(in_feat, W, b, src, dst):
    full, _ = run(in_feat, W, b, src, dst)
    return full


# revision 36
# speedup vs baseline: 1.0778x; 1.0778x over previous
"""APPNP GNN kernel for 8 Trainium2 NeuronCores.

h = 0.8 * D_in^{-1/2} A D_out^{-1/2} (X W^T + b) + 0.2 * (X W^T + b)

Strategy: dst-shard nodes across 8 cores. Each core computes h0 for its
own nodes (PE matmul, bf16), scales by rsqrt(out-degree), AllGathers the
bf16 table in 4 chunks, then per-edge dma_gathers source rows from HBM
and segment-reduces them into per-dst-window PSUM tiles via one-hot
matmuls (S^T @ msgs).

Pipeline decoupling: passes run pass-major, window-major within each
pass, with pass 0 overlapping phase 1 / the AllGathers. Each pass has
its own bf16 accumulator slab; finished window psums are drained there
by the SCALAR engine only, so no mid-stream op on Vector ever waits on
fresh matmul results -- Vector does nothing but the batched IS_EQ
one-hot builds and runs ahead. A single batched endgame sums the four
accumulators, applies norm_in, and blends into 0.2*h0 (all bf16;
output cast to fp32 on host).

Perf note (measured): the wall is the per-edge dma_gather descriptor
stream itself -- gather-only is ~1.95 ms of the ~2.04 ms total.
Per-descriptor cost has a ~2.6 ns/desc aggregate mechanics floor
(sequential-index probe: 1.08 ms) plus a random-access HBM latency
adder (+0.87 ms). SBUF-source transpose-mode gathers measured
~3.35 ns/desc (1.40 ms) but need a per-tile PE re-transpose that
wrecks the pipeline. Descriptor count = E/core is irreducible, so
~1.95 ms is the practical floor for this gather-based design.
"""

import os
import sys

sys.path.insert(0, "/opt/trn_rl_repo")

SKIP_EDGES = os.environ.get("K_SKIP_EDGES", "") == "1"
GATHER_ONLY = os.environ.get("K_GATHER_ONLY", "") == "1"
FAKE_IDX = os.environ.get("K_FAKE_IDX", "") == "1"
SBUF_GATHER_PROBE = os.environ.get("K_SBUF_PROBE", "") == "1"
SINGLE_PACKET = os.environ.get("K_SINGLE_PACKET", "0") == "1"
SLIPS = tuple(
    int(x) for x in os.environ.get("K_SLIPS", "3,5,7").split(",")
)
SBATCH = int(os.environ.get("K_SBATCH", "16"))  # S one-hots per DVE op
WG = int(os.environ.get("K_WG", "16"))  # windows per psum group
PSW = 8  # windows per PSUM bank tile
GPOOL = int(os.environ.get("K_GPOOL", "7"))
SPOOL = int(os.environ.get("K_SPOOL", "4"))
NQUEUE = int(os.environ.get("K_NQUEUE", "4"))

import numpy as np
import ml_dtypes

import concourse.bass as bass
import concourse.bacc as bacc
import concourse.tile as tile
import concourse.mybir as mybir
from concourse.bass_utils import run_bass_kernel_spmd

F32 = mybir.dt.float32
BF16 = mybir.dt.bfloat16
I16 = mybir.dt.int16
I32 = mybir.dt.int32

NCORES = 8
SLAB_TILES = int(os.environ.get("K_SLAB", "32"))  # max edge tiles per dma_gather
CHT = 256        # idx chunk size in tiles
ALPHA = 0.2
TPB = 4  # node tiles per phase-1 matmul group


def _cfg(N, F, C):
    sh = (N + NCORES - 1) // NCORES          # nodes per core
    shp = ((sh + 127) // 128) * 128          # padded to 128
    nw = shp // 128                          # dst windows per core
    # split each core's shard into NPASS quarters (tile-aligned to TPB) so
    # AllGather q can fire as soon as phase 1 finishes quarter q
    npass = 4
    # small starter quarter: AllGather 0 (and the first gathers) launch
    # as soon as the first 8 node tiles of phase 1 are done
    q0 = 8
    base = (nw - q0) // (npass - 1)
    q_tiles = [q0] + [base] * (npass - 2) + [nw - q0 - base * (npass - 2)]
    q_rows = [t * 128 for t in q_tiles]
    q_off = list(np.cumsum([0] + q_rows[:-1]))
    chunk_rows = [NCORES * r for r in q_rows]          # table rows per chunk
    assert max(chunk_rows) < 32768, chunk_rows
    return dict(N=N, F=F, C=C, SH=sh, SHP=shp, NW=nw, NPASS=npass,
                CHUNK_ROWS=chunk_rows, Q_TILES=q_tiles, Q_ROWS=q_rows,
                Q_OFF=q_off)


def _tile_schedule(t_pw, NW, NPASS):
    """Stream order: pass-major, window-major within pass. Returns
    tile_order [(p, w)], seg_pos {(p,w): first tile}, and the slab
    list [(t0, nts, pass)]."""
    tile_order = []
    for p in range(NPASS):
        for w in range(NW):
            tile_order += [(p, w)] * int(t_pw[p][w])
    ntiles = len(tile_order)
    seg_pos = {}
    prev = None
    for t, pw in enumerate(tile_order):
        if pw != prev:
            seg_pos[pw] = t
            prev = pw
    slabs = []
    t0 = 0
    while t0 < ntiles:
        p = tile_order[t0][0]
        end = min(t0 + SLAB_TILES, ntiles, (t0 // CHT + 1) * CHT)
        e = t0
        while e < end and tile_order[e][0] == p:
            e += 1
        slabs.append((t0, e - t0, p))
        t0 = e
    return tile_order, seg_pos, slabs


def _host_prep(in_feat, W, b, src, dst, cfg):
    """Shard + reformat inputs; build core-uniform edge-tile structure."""
    N, F, C = cfg["N"], cfg["F"], cfg["C"]
    SH, SHP, NW, NPASS = cfg["SH"], cfg["SHP"], cfg["NW"], cfg["NPASS"]

    src = np.asarray(src, dtype=np.int64)
    dst = np.asarray(dst, dtype=np.int64)

    # chunk q = quarter q of EVERY core's shard (so AllGather q only needs
    # phase-1 quarter q); within chunk q rows are core-major
    q_rows = np.asarray(cfg["Q_ROWS"], dtype=np.int64)
    q_off = np.asarray(cfg["Q_OFF"], dtype=np.int64)
    score = src // SH
    slocal = src % SH
    passno = (np.searchsorted(q_off, slocal, side="right") - 1).astype(np.int64)
    idx16 = (score * q_rows[passno] + (slocal - q_off[passno])).astype(np.int32)
    passno = passno.astype(np.int32)
    core = (dst // SH).astype(np.int32)
    dloc = (dst % SH).astype(np.int64)
    wno = (dloc // 128).astype(np.int32)
    drel = (dloc % 128).astype(np.int32)

    # per-(core, pass, window) counts -> uniform tile structure
    key = (core.astype(np.int64) * NPASS + passno) * NW + wno
    counts = np.bincount(key, minlength=NCORES * NPASS * NW).reshape(
        NCORES, NPASS, NW
    )
    t_pw = np.maximum((counts + 127) // 128, 1).max(axis=0)  # [NPASS, NW]

    tile_order, seg_pos, slabs = _tile_schedule(t_pw, NW, NPASS)
    ntiles = len(tile_order)

    # per-core streams
    in_feat = np.asarray(in_feat, dtype=np.float32)
    WT = np.ascontiguousarray(
        np.asarray(W, dtype=np.float32).T.astype(ml_dtypes.bfloat16)
    )  # [F, C] bf16
    bias = np.asarray(b, dtype=np.float32).reshape(C, 1)
    ident = np.eye(C, dtype=np.float32)

    deg_out = np.bincount(src, minlength=N)
    deg_in = np.bincount(dst, minlength=N)
    rp_out = np.concatenate([[0], np.cumsum(deg_out)])
    rp_in = np.concatenate([[0], np.cumsum(deg_in)])

    iota = np.tile(np.arange(128, dtype=np.float32), (128, SBATCH)).astype(
        ml_dtypes.bfloat16
    )  # [128, SBATCH*128]

    order = np.lexsort((idx16, wno, passno, core))
    so_pass, so_idx, so_rel = passno[order], idx16[order], drel[order]
    so_core, so_w = core[order], wno[order]
    # edge range per (core, pass, window) in sorted order
    seg_start = np.searchsorted(
        (so_core.astype(np.int64) * NPASS + so_pass) * NW + so_w,
        np.arange(NCORES * NPASS * NW + 1),
    ).reshape(-1)

    NBLK = (NW + TPB - 1) // TPB
    in_maps = []
    for k in range(NCORES):
        inT = np.zeros((F, SHP), dtype=np.float32)
        lo, hi = k * SH, min((k + 1) * SH, N)
        inT[:, : hi - lo] = in_feat[lo:hi].T
        # [p, blk, c, n]: partition-contiguous per phase-1 block DMA
        inT_p = np.zeros((F, NBLK * TPB * 128), dtype=np.float32)
        inT_p[:, :SHP] = inT
        v = inT_p.reshape(F // 128, 128, NBLK, TPB * 128)
        inT_t = np.ascontiguousarray(v.transpose(1, 2, 0, 3)).astype(
            ml_dtypes.bfloat16
        )

        def rp_mats(rp):
            v = rp[lo : hi + 1]
            v = np.concatenate([v, np.full(SHP + 1 - len(v), v[-1], v.dtype)])
            lo_m = v[:SHP].reshape(NW, 128).T.astype(np.int32)
            hi_m = v[1 : SHP + 1].reshape(NW, 128).T.astype(np.int32)
            return np.ascontiguousarray(lo_m), np.ascontiguousarray(hi_m)

        rpo_lo, rpo_hi = rp_mats(rp_out)
        rpi_lo, rpi_hi = rp_mats(rp_in)

        idx_stream = np.zeros(ntiles * 128, dtype=np.int16)
        rel_stream = np.full(ntiles * 128, -1.0, dtype=np.float32)
        for p in range(NPASS):
            for w in range(NW):
                s0 = seg_start[(k * NPASS + p) * NW + w]
                s1 = seg_start[(k * NPASS + p) * NW + w + 1]
                off = seg_pos[(p, w)] * 128
                idx_stream[off : off + (s1 - s0)] = so_idx[s0:s1]
                rel_stream[off : off + (s1 - s0)] = so_rel[s0:s1]

        if FAKE_IDX:
            # sequential indices: perfect-locality gather experiment
            idx_stream = (np.arange(ntiles * 128) % 24576).astype(np.int16)
        if SBUF_GATHER_PROBE:
            rng = np.random.RandomState(0)
            idx_stream = rng.randint(
                0, 12544, size=ntiles * 128
            ).astype(np.int16)
        idx_w = np.tile(
            np.ascontiguousarray(idx_stream.reshape(-1, 16).T), (8, 1)
        )  # [128, ntiles*8]: 16-part wrap replicated per Q7 core
        rel_m = np.ascontiguousarray(
            rel_stream.reshape(ntiles, 128).T.astype(ml_dtypes.bfloat16)
        )  # [128, ntiles]

        in_maps.append(
            {
                "inT": inT_t,
                "wt": WT,
                "bias": bias,
                "ident": ident,
                "iota": iota,
                "rpo_lo": rpo_lo,
                "rpo_hi": rpo_hi,
                "rpi_lo": rpi_lo,
                "rpi_hi": rpi_hi,
                "idx": idx_w,
                "rel": rel_m,
            }
        )

    struct = dict(t_pw=t_pw, ntiles=ntiles, tile_order=tile_order,
                  seg_pos=seg_pos, slabs=slabs)
    return in_maps, struct


def _build_program(cfg, struct):
    F, C = cfg["F"], cfg["C"]
    SHP, NW, NPASS = cfg["SHP"], cfg["NW"], cfg["NPASS"]
    CHUNK_ROWS = cfg["CHUNK_ROWS"]
    t_pw, ntiles = struct["t_pw"], struct["ntiles"]
    tile_order, seg_pos, slabs = (
        struct["tile_order"], struct["seg_pos"], struct["slabs"],
    )
    KC = F // 128  # contraction chunks in phase 1

    nc = bacc.Bacc(
        "TRN2", target_bir_lowering=False, debug=False, num_devices=NCORES,
        num_swdge_queues=4,
    )

    NBLK = (NW + 3) // 4
    inT_d = nc.dram_tensor(
        "inT", [128, NBLK, F // 128, 4 * 128], BF16, kind="ExternalInput"
    ).ap()
    wt_d = nc.dram_tensor("wt", [F, C], BF16, kind="ExternalInput").ap()
    bias_d = nc.dram_tensor("bias", [C, 1], F32, kind="ExternalInput").ap()
    ident_d = nc.dram_tensor("ident", [C, C], F32, kind="ExternalInput").ap()
    iota_d = nc.dram_tensor(
        "iota", [128, SBATCH * 128], BF16, kind="ExternalInput"
    ).ap()
    rp_d = {
        n: nc.dram_tensor(n, [128, NW], I32, kind="ExternalInput").ap()
        for n in ("rpo_lo", "rpo_hi", "rpi_lo", "rpi_hi")
    }
    idx_d = nc.dram_tensor(
        "idx", [128, ntiles * 8], I16, kind="ExternalInput"
    ).ap()
    rel_d = nc.dram_tensor("rel", [128, ntiles], BF16, kind="ExternalInput").ap()
    hout_d = nc.dram_tensor("hout", [SHP, C], BF16, kind="ExternalOutput").ap()

    with tile.TileContext(nc) as tc:
        with (
            tc.tile_pool(name="const", bufs=1) as cpool,
            tc.tile_pool(name="bigbuf", bufs=1) as bpool,
            tc.tile_pool(name="inT", bufs=3) as ipool,
            tc.tile_pool(name="gat", bufs=GPOOL) as gpool,
            tc.tile_pool(name="idxs", bufs=3) as idxpool,
            tc.tile_pool(name="sbuild", bufs=SPOOL) as spool,
            tc.tile_pool(name="ps1", bufs=2, space="PSUM") as ps1,
            tc.tile_pool(name="pse", bufs=4, space="PSUM") as pse,
            tc.tile_pool(name="dram", bufs=1, space="DRAM") as dpool,
        ):
            # ---- load constants ----
            wt_s = cpool.tile([128, KC, C], BF16, tag="wt")
            nc.sync.dma_start(
                wt_s[:], wt_d.rearrange("(c p) f -> p c f", p=128)
            )
            bias_s = cpool.tile([C, 1], F32, tag="bias")
            nc.sync.dma_start(bias_s[:], bias_d)
            ident_s = cpool.tile([C, C], F32, tag="ident")
            nc.sync.dma_start(ident_s[:], ident_d)
            iota_s = cpool.tile([128, SBATCH * 128], BF16, tag="iota")
            nc.sync.dma_start(iota_s[:], iota_d)
            rel_s = cpool.tile([128, ntiles], BF16, tag="rel")
            nc.sync.dma_start(rel_s[:], rel_d)
            rp_s = {}
            for n in rp_d:
                rp_s[n] = cpool.tile([128, NW], I32, tag=n, name=n)
                nc.sync.dma_start(rp_s[n][:], rp_d[n])

            # ---- degree norms: norm = sqrt(scale / clip(deg, 1)) ----
            def make_norm(lo, hi, scale, tag):
                deg = cpool.tile([128, NW], F32, tag=tag + "_deg")
                nc.vector.tensor_tensor(
                    deg[:], hi[:], lo[:], op=mybir.AluOpType.subtract
                )
                nc.vector.tensor_scalar_max(deg[:], deg[:], 1.0)
                rec = cpool.tile([128, NW], F32, tag=tag + "_rec")
                nc.vector.reciprocal(rec[:], deg[:])
                norm = cpool.tile([128, NW], F32, tag=tag)
                nc.scalar.activation(
                    norm[:], rec[:], mybir.ActivationFunctionType.Sqrt,
                    scale=scale,
                )
                return norm

            norm_out = make_norm(rp_s["rpo_lo"], rp_s["rpo_hi"], 1.0, "nout")
            norm_in = make_norm(
                rp_s["rpi_lo"], rp_s["rpi_hi"], (1.0 - ALPHA) ** 2, "nin"
            )

            # ---- big SBUF buffers ----
            h0s_s = bpool.tile([128, NW, 128], BF16, tag="h0s")  # padded table
            nc.vector.memset(h0s_s[:, :, C:128], 0.0)
            h0b_s = bpool.tile([128, NW, C], BF16, tag="h0b")  # 0.2*h0 -> out
            acc4_s = bpool.tile([128, NPASS * NW, C], BF16, tag="acc4")

            # ---- phase 1 ----
            coll_ins = []
            h0s_dram = dpool.tile([SHP, 128], BF16, tag="h0s_dram",
                                  name="h0s_dram")
            tables = [
                dpool.tile([CHUNK_ROWS[q], 128], BF16, tag=f"table{q}",
                           name=f"table{q}", addr_space="Shared")
                for q in range(NPASS)
            ]
            Q_TILES = cfg["Q_TILES"]
            qt_off = list(np.cumsum([0] + Q_TILES[:-1]))

            qwrite_ins = {}

            def emit_quarter_write(q):
                a, b = qt_off[q], qt_off[q] + Q_TILES[q]
                di = nc.sync.dma_start(
                    h0s_dram[a * 128:b * 128].rearrange(
                        "(t p) f -> p t f", p=128
                    ),
                    h0s_s[:, a:b, :],
                )
                qwrite_ins[q] = di

            def emit_quarter_coll(q):
                a, b = qt_off[q], qt_off[q] + Q_TILES[q]
                ci = nc.gpsimd.collective_compute(
                    "AllGather",
                    mybir.AluOpType.bypass,
                    replica_groups=[list(range(NCORES))],
                    ins=[h0s_dram[a * 128:b * 128].opt()],
                    outs=[tables[q][:].opt()],
                )
                # DRAM-space deps are not auto-tracked: explicitly order the
                # collective after its quarter's h0s_dram write completes
                tile.add_dep_helper(ci.ins, qwrite_ins[q].ins, sync=True)
                coll_ins.append(ci.ins)

            def emit_phase1():
                nq = 0
                for g0 in range(0, NW, TPB):
                    nb = min(TPB, NW - g0)
                    t = ipool.tile([128, KC, TPB * 128], BF16, tag="inT",
                                   name="t")
                    nc.sync.dma_start(t[:], inT_d[:, g0 // TPB, :, :])
                    psT = ps1.tile([C, TPB * 128], F32, tag="psT", name="psT")
                    for c in range(KC):
                        nc.tensor.matmul(
                            psT[:, : nb * 128],
                            lhsT=wt_s[:, c, :],
                            rhs=t[:, c, : nb * 128],
                            start=(c == 0),
                            stop=(c == KC - 1),
                        )
                    h0T = ipool.tile([C, TPB * 128], F32, tag="h0T",
                                     name="h0T")
                    nc.vector.tensor_scalar(
                        h0T[:, : nb * 128], psT[:, : nb * 128], bias_s[:],
                        None, op0=mybir.AluOpType.add,
                    )
                    for j in range(nb):
                        tt = g0 + j
                        pst = ps1.tile([128, C], F32, tag="pst", name="pst")
                        nc.tensor.transpose(
                            pst[:], h0T[:, j * 128 : (j + 1) * 128], ident_s[:]
                        )
                        nc.scalar.activation(
                            h0s_s[:, tt, 0:C], pst[:],
                            mybir.ActivationFunctionType.Copy,
                            scale=norm_out[:, tt : tt + 1],
                        )
                        nc.scalar.activation(
                            h0b_s[:, tt, :], pst[:],
                            mybir.ActivationFunctionType.Copy, scale=ALPHA,
                        )
                    while nq < NPASS and g0 + nb >= qt_off[nq] + Q_TILES[nq]:
                        emit_quarter_write(nq)
                        if nq == 0:
                            emit_quarter_coll(0)
                        nq += 1
                assert nq == NPASS, (nq, NPASS)

            # ---- SBUF-gather probe: copy part of chunk 0 into SBUF
            # and run transpose-mode gathers from it ----
            tbl_sbuf = None
            if SBUF_GATHER_PROBE:
                tbl_sbuf = bpool.tile([128, 16384], BF16, tag="tblsb")
                nc.sync.dma_start(
                    tbl_sbuf[:, : 98 * 128],
                    tables[0][0:12544].rearrange(
                        "(p t) f -> p (t f)", p=128
                    ),
                )

            # ---- edge phase ----
            st = dict(s_cur=None, nslab=0)
            ps_by_w = {}
            seen_tbl = set()

            def load_idx_chunk(c0):
                # chunk covers tiles [c0, c0+CHT)
                it = idxpool.tile([128, CHT * 8], I16, tag="idxs",
                                  name=f"it{c0}")
                ncols = min(CHT, ntiles - c0) * 8
                nc.sync.dma_start(
                    it[:, :ncols], idx_d[:, c0 * 8 : c0 * 8 + ncols]
                )
                return it

            idx_chunks = {}
            if not SKIP_EDGES:
                idx_chunks[0] = load_idx_chunk(0)
                if ntiles > CHT:
                    idx_chunks[CHT] = load_idx_chunk(CHT)


            def emit_slab(si, t0, nts, p):
                tbl = tables[p][:]
                # idx chunk management + prefetch 2 ahead
                c0 = (t0 // CHT) * CHT
                nxt = c0 + 2 * CHT
                if nxt < ntiles and nxt not in idx_chunks:
                    idx_chunks[nxt] = load_idx_chunk(nxt)
                it = idx_chunks[c0]
                if p == 0 and si in SLIPS:
                    q = SLIPS.index(si) + 1
                    if len(coll_ins) == q:
                        emit_quarter_coll(q)
                if SBUF_GATHER_PROBE:
                    g = gpool.tile([128, 1, SLAB_TILES * 128], BF16,
                                   tag="gat", name="g")
                else:
                    g = gpool.tile([128, SLAB_TILES, 128], BF16, tag="gat",
                                   name="g")
                nidx = nts * 128
                toff = t0 - c0
                if SBUF_GATHER_PROBE:
                    gi = nc.gpsimd.dma_gather(
                        g[:, :, :nidx],
                        tbl_sbuf[:],
                        it[:, toff * 8 : toff * 8 + nidx // 16],
                        num_idxs=nidx,
                        num_idxs_reg=nidx,
                        elem_size=128,
                        transpose=True,
                        single_packet=SINGLE_PACKET,
                        queue_num=st["nslab"] % NQUEUE,
                        sbuf_tokens_per_rank=128,
                        sbuf_free_dim_per_rank=32768,
                    )
                else:
                    gi = nc.gpsimd.dma_gather(
                        g[:, :nts, :],
                        tbl,
                        it[:, toff * 8 : toff * 8 + nidx // 16],
                        num_idxs=nidx,
                        num_idxs_reg=nidx,
                        elem_size=128,
                        single_packet=SINGLE_PACKET,
                        queue_num=st["nslab"] % NQUEUE,
                    )
                st["nslab"] += 1
                if p not in seen_tbl:
                    seen_tbl.add(p)
                    tile.add_dep_helper(gi.ins, coll_ins[p], sync=True)
                if not GATHER_ONLY:
                    for ti in range(nts):
                        emit_tile(t0, ti, p, g)

            def emit_tile(t0, ti, p, g):
                tt = t0 + ti
                if tt % SBATCH == 0:  # build S batch
                    st["s_cur"] = spool.tile(
                        [128, SBATCH * 128], BF16, tag="sb", name="s"
                    )
                    nb4 = min(SBATCH, ntiles - tt)
                    rel_b = rel_s[:, tt : tt + nb4].unsqueeze(-1)
                    nc.vector.tensor_tensor(
                        st["s_cur"][:, : nb4 * 128].rearrange(
                            "p (a b) -> p a b", b=128
                        ),
                        iota_s[:, : nb4 * 128].rearrange(
                            "p (a b) -> p a b", b=128
                        ),
                        rel_b.broadcast_to((128, nb4, 128)),
                        op=mybir.AluOpType.is_equal,
                    )
                sc = tt % SBATCH
                s_t = st["s_cur"][:, sc * 128 : (sc + 1) * 128]
                pw, w = tile_order[tt]
                assert pw == p
                tloc = tt - seg_pos[(p, w)]
                first = tloc == 0
                last = tloc == t_pw[p][w] - 1
                if first:
                    ps_by_w[w] = pse.tile([128, C], F32, tag="pse",
                                          name="ps")
                cur_ps = ps_by_w.pop(w) if last else ps_by_w[w]
                nc.tensor.matmul(
                    cur_ps[:],
                    lhsT=s_t,
                    rhs=g[:, ti, 0:C],
                    start=first,
                    stop=last,
                )
                if last:
                    # drain psum to this pass's bf16 accumulator (scalar
                    # engine only -- no mid-stream vector dependency)
                    nc.scalar.activation(
                        acc4_s[:, p * NW + w, :], cur_ps[:],
                        mybir.ActivationFunctionType.Copy,
                    )

            norm_bf = cpool.tile([128, NW], BF16, tag="nin_bf")
            nc.vector.tensor_copy(norm_bf[:], norm_in[:])

            emit_phase1()
            if not SKIP_EDGES:
                si = -1
                for t0, nts, p in slabs:
                    if p == 0:
                        si += 1
                    else:
                        while len(coll_ins) <= p:
                            emit_quarter_coll(len(coll_ins))
                    emit_slab(si if p == 0 else -1, t0, nts, p)
                # endgame: batched reduce of the 4 accumulators + blend
                a = acc4_s.rearrange("p (q w) f -> p q w f", q=NPASS)
                nc.vector.tensor_tensor(
                    a[:, 0], a[:, 0], a[:, 1], op=mybir.AluOpType.add
                )
                nc.vector.tensor_tensor(
                    a[:, 2], a[:, 2], a[:, 3], op=mybir.AluOpType.add
                )
                nc.vector.tensor_tensor(
                    a[:, 0], a[:, 0], a[:, 2], op=mybir.AluOpType.add
                )
                nb = norm_bf[:].unsqueeze(-1)
                nc.vector.tensor_tensor(
                    a[:, 0], a[:, 0], nb.broadcast_to((128, NW, C)),
                    op=mybir.AluOpType.mult,
                )
                nc.vector.tensor_tensor(
                    h0b_s[:], h0b_s[:], a[:, 0], op=mybir.AluOpType.add
                )
                nc.scalar.dma_start(
                    hout_d.rearrange("(t p) f -> p t f", p=128), h0b_s[:]
                )
            else:
                for q in range(1, NPASS):
                    emit_quarter_coll(q)
                nc.sync.dma_start(
                    hout_d.rearrange("(t p) f -> p t f", p=128), h0b_s[:]
                )
            if GATHER_ONLY:
                nc.sync.dma_start(
                    hout_d.rearrange("(t p) f -> p t f", p=128), h0b_s[:]
                )

    nc.compile()
    return nc


def run(in_feat, W, b, src, dst, trace=False):
    N, F = in_feat.shape
    C = W.shape[0]
    cfg = _cfg(N, F, C)
    in_maps, struct = _host_prep(in_feat, W, b, src, dst, cfg)
    nc = _build_program(cfg, struct)
    res = run_bass_kernel_spmd(
        nc, in_maps, list(range(NCORES)), trace=trace
    )
    outs = [res.results[k]["hout"][: cfg["SH"]] for k in range(NCORES)]
    full = np.concatenate(outs, axis=0)[:N].astype(np.float32)
    return full, res


def kernel(in_feat, W, b, src, dst):
    full, _ = run(in_feat, W, b, src, dst)
    return full
